# revision 11
# baseline (speedup 1.0000x reference)
"""Adaptive average pooling 2D on 8 TRN2 NeuronCores.

Input  x: (16, 224, 224, 128) f32 channels_last -> output (16, 7, 7, 128) f32.
Since 224 = 7*32 the adaptive bins are uniform 32x32 windows:
out[b,i,j,c] = mean over the 32x32 spatial block (i,j) of sample b.

Sharding: data parallel over batch -> 2 samples per core, no communication.

Per-core kernel (raw bacc, manual semaphores; x viewed as [448, 28672] rows):
  - 4 full-row SWDGE cast-DMAs (f32 DRAM -> bf16 SBUF) of ~11-14.7MB each;
    114KB contiguous DRAM reads per partition line run near HBM line rate.
    The last load is split into quarters so the PE tail after the final
    packet stays short.
  - h-reduction on the TensorEngine: block-diagonal lhsT [K,4] (1/1024 on
    32-row blocks, bf16) contracts 128/96 rows per chunk; 8 matmuls per
    32x32 window accumulate the w-chunks into one [4,512] PSUM bank
    (8 banks rotate).
  - remaining 4-way strided w-sum on the VectorEngine (PSUM -> SBUF),
    collected in one [4, 3584] tile; 2 strided HWDGE DMAs write the output.
"""

import numpy as np

B, H, W, C = 16, 224, 224, 128
NCORES = 8
BPC = B // NCORES  # samples per core
OUT_H = OUT_W = 7
BLK = 32
ROWC = W * C  # 28672 contiguous f32 per (b, h) row
H_CHUNKS = ((0, 128, 4), (128, 96, 3))  # (row0, K, M) per h-chunk
INV_AREA = 1.0 / float(BLK * BLK)
QW = ROWC // 4

_NC = None


def _weight_f32() -> np.ndarray:
    w = np.zeros((128, 4), dtype=np.float32)
    for m in range(4):
        w[32 * m:32 * m + 32, m] = INV_AREA
    return w


def _build_nc():
    import concourse.bacc as bacc
    import concourse.mybir as mybir
    from contextlib import ExitStack

    f32 = mybir.dt.float32
    bf16 = mybir.dt.bfloat16
    nc = bacc.Bacc("TRN2", target_bir_lowering=False, debug=False,
                   enable_asserts=False)
    x_ext = nc.dram_tensor("x", [BPC * H, ROWC], f32, kind="ExternalInput")
    w_ext = nc.dram_tensor("w", [128, 4], bf16, kind="ExternalInput")
    out_ext = nc.dram_tensor("out", [BPC * OUT_H, OUT_W * C], f32,
                             kind="ExternalOutput")
    iters = [(b, hc) for b in range(BPC) for hc in range(2)]
    NB = 8  # rotating psum banks

    with ExitStack() as ctx:
        wtile = ctx.enter_context(nc.sbuf_tensor("wtile", [128, 4], bf16))
        slots = [ctx.enter_context(
                     nc.sbuf_tensor(f"slot{p_}", [128, ROWC], bf16))
                 for p_ in range(2)]
        otile = ctx.enter_context(
            nc.sbuf_tensor("otile", [4, 2 * BPC * OUT_W * C], f32))
        psum = [ctx.enter_context(nc.psum_tensor(f"psum{i}", [4, 512], f32))
                for i in range(NB)]
        wsem = ctx.enter_context(nc.semaphore("wsem"))
        insem = [ctx.enter_context(nc.semaphore(f"insem{i}"))
                 for i in range(4)]
        qsem = [ctx.enter_context(nc.semaphore(f"qsem{i}"))
                for i in range(4)]
        pesem = ctx.enter_context(nc.semaphore("pesem"))
        dvesem = ctx.enter_context(nc.semaphore("dvesem"))
        osem = ctx.enter_context(nc.semaphore("osem"))
        block = ctx.enter_context(nc.Block())

        @block.sync
        def _(sync):
            sync.dma_start(out=wtile[:, :], in_=w_ext[:, :]).then_inc(
                wsem, 16)
            dview = out_ext[:, :].rearrange(
                "(b i) (j c) -> i b j c", b=BPC, j=OUT_W)
            # reduce groups run in order (it0,it1,it2,it3) = (hc0,hc1,hc0,hc1)
            # so hc0's rows are complete after it2's reduces (dvesem>=21).
            for hc, thresh in ((0, 3 * OUT_W), (1, 4 * OUT_W)):
                M = H_CHUNKS[hc][2]
                sync.wait_ge(dvesem, thresh)
                sl = otile[:M, hc * BPC * OUT_W * C:(hc + 1) * BPC * OUT_W * C]
                sync.dma_start(
                    out=dview[hc * 4:hc * 4 + M],
                    in_=sl.rearrange("m (b j c) -> m b j c", b=BPC, j=OUT_W),
                ).then_inc(osem, 16)
            sync.wait_ge(osem, 32)

        @block.gpsimd
        def _(gpsimd):
            for it, (b, hc) in enumerate(iters):
                r0, K, M = H_CHUNKS[hc]
                row0 = b * H + r0
                t = slots[it % 2]
                if it >= 2:
                    # slot reuse: all matmul groups of it-2 must be done
                    gpsimd.wait_ge(pesem, OUT_W * (it - 1))
                if it == len(iters) - 1:
                    for q in range(4):
                        gpsimd.dma_start(
                            out=t[:K, q * QW:(q + 1) * QW],
                            in_=x_ext[row0:row0 + K, q * QW:(q + 1) * QW],
                        ).then_inc(qsem[q], 16)
                else:
                    gpsimd.dma_start(
                        out=t[:K, :], in_=x_ext[row0:row0 + K, :],
                    ).then_inc(insem[it], 16)
            gpsimd.wait_ge(qsem[3], 16)

        @block.tensor
        def _(tensor):
            tensor.wait_ge(wsem, 16)
            g = 0
            for it, (b, hc) in enumerate(iters):
                r0, K, M = H_CHUNKS[hc]
                last = it == len(iters) - 1
                t = slots[it % 2]
                for j in range(OUT_W):
                    if not last:
                        if j == 0:
                            tensor.wait_ge(insem[it], 16)
                    else:
                        # quarter q covers w in [56q, 56q+56)
                        if j == 0:
                            tensor.wait_ge(qsem[0], 16)
                        elif j == 1:
                            tensor.wait_ge(qsem[1], 16)
                        elif j == 3:
                            tensor.wait_ge(qsem[2], 16)
                    if g >= NB:
                        tensor.wait_ge(dvesem, g - NB + 1)
                    p = psum[g % NB]
                    for k in range(8):
                        w0 = BLK * j + 4 * k
                        if last and j == 5 and k == 2:
                            tensor.wait_ge(qsem[3], 16)
                        ins = tensor.matmul(
                            p.ap()[:M, :],
                            wtile[:K, :M],
                            t[:K, w0 * C:w0 * C + 512],
                            start=(k == 0), stop=(k == 7))
                        if k == 7:
                            ins.then_inc(pesem, 1)
                    g += 1

        @block.vector
        def _(vector):
            g = 0
            for it, (b, hc) in enumerate(iters):
                r0, K, M = H_CHUNKS[hc]
                for j in range(OUT_W):
                    off_o = ((hc * BPC + b) * OUT_W + j) * C
                    vector.wait_ge(pesem, g + 1)
                    vector.tensor_reduce(
                        otile[:M, off_o:off_o + C],
                        psum[g % NB].ap()[:M, :].rearrange(
                            "p (u c) -> p c u", u=4),
                        axis=mybir.AxisListType.X,
                        op=mybir.AluOpType.add,
                    ).then_inc(dvesem, 1)
                    g += 1

    nc.compile()
    return nc


def _get_nc():
    global _NC
    if _NC is None:
        _NC = _build_nc()
    return _NC


def _in_maps(x: np.ndarray):
    import ml_dtypes
    w = _weight_f32().astype(ml_dtypes.bfloat16)
    return [
        {"x": x[BPC * c:BPC * (c + 1)].reshape(BPC * H, ROWC), "w": w}
        for c in range(NCORES)
    ]


def kernel(x: np.ndarray) -> np.ndarray:
    from concourse.bass_utils import run_bass_kernel_spmd

    nc = _get_nc()
    x = np.ascontiguousarray(np.asarray(x, dtype=np.float32))
    assert x.shape == (B, H, W, C)
    res = run_bass_kernel_spmd(nc, _in_maps(x), core_ids=list(range(NCORES)))
    outs = [r["out"].reshape(BPC, OUT_H, OUT_W, C) for r in res.results]
    return np.concatenate(outs, axis=0)
